# revision 2
# baseline (speedup 1.0000x reference)
"""Blockwise-quant linear (fp8 e4m3fn weights + per-(row,128-block) activation
quant) as a Trainium2 Bass/Tile kernel, row-parallel over 8 NeuronCores.

y[m,n] = sum_k xd[m,k] * wd[n,k], where
  xd = e4m3fn_round(x / a_s) * a_s,  a_s[m,kb] = max(amax128(x), 1e-4)/448
  wd = fp8_weight * w_scale[nb,kb]

Sharding: rows of x (M) split across cores; weight/w_scale replicated.
Each core computes y[1024, 4096] f32; host concatenates.

Device fp8 is IEEE e4m3 (max 240), reference uses e4m3fn (max 448):
 - weight bytes reinterpret exactly (values never reach exp-field-15),
 - activation quant uses half-scale: e4m3fn(v) == 2*e4m3(v/2) for |v|>2^-5.

v2 structure (single fused pipeline, PE near-saturated):
 - x-path per m-tile: DMA -> amax/scale/quant (DVE) -> dequant (ACT,
   per-partition scale per k-block) -> X-bar DMA-transpose into resident
   xdT[k, kb, m] (no PE transposes, no PSUM drains on the x path).
 - GEMM for chunk-pair 0 is software-pipelined one m-tile behind the
   x-path, so the PE starts ~12us in instead of after the whole x phase.
 - Accumulation is kb-outer/ch-inner: 2 matmuls per stationary load into
   2 PSUM banks; 8 acc banks rotate across m-tiles.
 - Weight dequant (DVE/GpSimd alternating) is emitted kb-major to match
   GEMM consumption order; later pairs overlap their own GEMM.
"""

import os
from contextlib import ExitStack

import ml_dtypes
import numpy as np

import concourse.bass as bass
import concourse.mybir as mybir
import concourse.tile as tile
from concourse import bacc
from concourse.bass_utils import run_bass_kernel_spmd
from concourse.masks import make_identity

M, K, N = 8192, 4096, 4096
B = 128                 # quant block
NCORES = 8
MS = M // NCORES        # 1024 rows of x per core
KB = K // B             # 32 k-blocks
NB = N // B             # 32 n-blocks
CW = 512                # matmul moving width (1 PSUM bank of f32)
NCH = N // CW           # 8 output column chunks per core
MT = MS // B            # 8 m-tiles per core
G = 4                   # x-path column groups per m-tile
GK = KB // G            # 8 k-blocks per group

F32 = mybir.dt.float32
BF16 = mybir.dt.bfloat16
FP8 = mybir.dt.float8e4

# "dma": X-bar DMA transpose for xd -> xdT; "pe": PE-transpose fallback
TRANSPOSE_MODE = os.environ.get("KBQ_TRANSPOSE", "dma")


def _kernel_body(tc, nc, x_in, w_in, s_in, y_out):
    pe_t = TRANSPOSE_MODE == "pe"
    with ExitStack() as ctx:
        consts = ctx.enter_context(tc.tile_pool(name="consts", bufs=1))
        xpool = ctx.enter_context(tc.tile_pool(name="xpool", bufs=2))
        spool = ctx.enter_context(tc.tile_pool(name="spool", bufs=2))
        xqpool = ctx.enter_context(tc.tile_pool(name="xqpool", bufs=2))
        xdpool = ctx.enter_context(tc.tile_pool(name="xdpool", bufs=2))
        xdtp = ctx.enter_context(tc.tile_pool(name="xdtp", bufs=1))
        wqpool = ctx.enter_context(tc.tile_pool(name="wqpool", bufs=12))
        wdpool = ctx.enter_context(tc.tile_pool(name="wdpool", bufs=2 * KB + 6))
        ypool = ctx.enter_context(tc.tile_pool(name="ypool", bufs=4))
        psum = ctx.enter_context(tc.tile_pool(name="psum", bufs=1, space="PSUM"))

        # w_scale, host-expanded to [128, KB, NB] (same value on every partition)
        ws_all = consts.tile([B, KB, NB], F32, name="ws_all")
        nc.gpsimd.dma_start(ws_all[:], s_in[:])

        if pe_t:
            identity = consts.tile([B, B], BF16, name="identity")
            make_identity(nc, identity)

        # resident dequantized-transposed activations: [128(k), kb, MS(m)]
        xdT = xdtp.tile([B, KB, MS], BF16, name="xdT")

        wds = {}
        weng = [nc.vector, nc.gpsimd]

        def emit_w(ch, kb, eng):
            wq = wqpool.tile([B, CW], FP8, name="wq", tag="wq")
            nc.sync.dma_start(wq[:], w_in[ch, kb])
            wd = wdpool.tile([B, CW], BF16, name="wd", tag="wd")
            eng.tensor_tensor(
                wd.rearrange("p (b j) -> p b j", j=B),
                wq.rearrange("p (b j) -> p b j", j=B),
                ws_all[:, kb, ch * (CW // B) : (ch + 1) * (CW // B)].broadcast_to(
                    [B, CW // B, B]
                ),
                op=mybir.AluOpType.mult,
            )
            wds[ch, kb] = wd

        def emit_w_range(pair, kbs):
            # kb-major, ch alternating: matches kb-outer GEMM consumption
            i = 0
            for kb in kbs:
                for ch in pair:
                    emit_w(ch, kb, weng[i % 2])
                    i += 1

        def emit_xpath(mt):
            ms = slice(mt * B, (mt + 1) * B)
            xnat = xpool.tile([B, K], BF16, name="xnat", tag="xnat")
            amax = spool.tile([B, KB], F32, name="amax", tag="amax")
            tsc = spool.tile([B, KB], F32, name="tsc", tag="tsc")
            r2 = spool.tile([B, KB], F32, name="r2", tag="r2")
            xq = xqpool.tile([B, K], FP8, name="xq", tag="xq")
            xd = xdpool.tile([B, K], BF16, name="xd", tag="xd")
            pend = None
            for g in range(G):
                gk = slice(g * GK, (g + 1) * GK)
                gq = slice(g * GK * B, (g + 1) * GK * B)
                nc.gpsimd.dma_start(xnat[:, gq], x_in[ms, gq])
                x3 = xnat[:, gq].rearrange("p (b j) -> p b j", j=B)
                nc.vector.tensor_reduce(
                    amax[:, gk], x3,
                    axis=mybir.AxisListType.X,
                    op=mybir.AluOpType.max,
                    apply_absolute_value=True,
                )
                # tsc = max(amax, 1e-4)/224  == 2*a_s (half-scale dequant scale)
                nc.vector.tensor_scalar(
                    tsc[:, gk], amax[:, gk], 1e-4, 1.0 / 224.0,
                    op0=mybir.AluOpType.max, op1=mybir.AluOpType.mult,
                )
                nc.vector.reciprocal(r2[:, gk], tsc[:, gk])
                # quantize the whole group in one op; fp8 RTNE on the store
                nc.vector.tensor_tensor(
                    xq[:, gq].rearrange("p (b j) -> p b j", j=B),
                    x3,
                    r2[:, gk].broadcast_to([B, GK, B]),
                    op=mybir.AluOpType.mult,
                )
                if pe_t:
                    # dequant whole group on GpSimd, PE-transpose + ACT drain
                    nc.gpsimd.tensor_tensor(
                        xd[:, gq].rearrange("p (b j) -> p b j", j=B),
                        xq[:, gq].rearrange("p (b j) -> p b j", j=B),
                        tsc[:, gk].broadcast_to([B, GK, B]),
                        op=mybir.AluOpType.mult,
                    )
                    pst = psum.tile([B, GK * B], BF16, name="pst", tag="pst", bufs=2)
                    for j in range(GK):
                        kb = g * GK + j
                        nc.tensor.transpose(
                            pst[:, j * B : (j + 1) * B],
                            xd[:, kb * B : (kb + 1) * B],
                            identity[:],
                        )
                    if pend is not None:
                        nc.scalar.copy(
                            pend[1], pend[0].rearrange("p (g j) -> p g j", j=B)
                        )
                    pend = (pst, xdT[:, gk, ms])
                else:
                    # dequant per k-block on ACT (per-partition scale), then
                    # X-bar DMA-transpose straight into resident xdT
                    for j in range(GK):
                        kb = g * GK + j
                        kc = slice(kb * B, (kb + 1) * B)
                        nc.scalar.mul(xd[:, kc], xq[:, kc], tsc[:, kb : kb + 1])
                        nc.sync.dma_start_transpose(xdT[:, kb, ms], xd[:, kc])
            if pend is not None:
                nc.scalar.copy(pend[1], pend[0].rearrange("p (g j) -> p g j", j=B))

        def emit_gemm(pair, mt):
            ms = slice(mt * B, (mt + 1) * B)
            accs = {
                ch: psum.tile([B, CW], F32, name="acc", tag="acc",
                              bufs=6 if pe_t else 8)
                for ch in pair
            }
            for kb in range(KB):
                for ch in pair:
                    nc.tensor.matmul(
                        accs[ch][:],
                        xdT[:, kb, ms],
                        wds[ch, kb][:],
                        start=(kb == 0),
                        stop=(kb == KB - 1),
                    )
            for ch in pair:
                yt = ypool.tile([B, CW], F32, name="yt", tag="yt")
                nc.scalar.copy(yt[:], accs[ch][:])
                nc.sync.dma_start(
                    y_out[ms, ch * CW : (ch + 1) * CW], yt[:]
                )

        # ---- phase 1: x-path pipelined with chunk-pair-0 GEMM ----
        for mt in range(MT):
            emit_xpath(mt)
            if mt == 0:
                emit_w_range((0, 1), range(0, KB // 2))
            elif mt == 1:
                emit_w_range((0, 1), range(KB // 2, KB))
            if mt >= 1:
                emit_gemm((0, 1), mt - 1)
        emit_gemm((0, 1), MT - 1)

        # ---- phase 2: remaining chunk pairs; weight dequant overlaps ----
        for cp in range(1, NCH // 2):
            pair = (2 * cp, 2 * cp + 1)
            emit_w_range(pair, range(KB))
            for mt in range(MT):
                emit_gemm(pair, mt)


def build():
    nc = bacc.Bacc(
        "TRN2", target_bir_lowering=False, debug=False, enable_asserts=False
    )
    x_in = nc.dram_tensor("x", (MS, K), BF16, kind="ExternalInput")
    w_in = nc.dram_tensor("wt", (NCH, KB, B, CW), FP8, kind="ExternalInput")
    s_in = nc.dram_tensor("ws", (B, KB, NB), F32, kind="ExternalInput")
    y_out = nc.dram_tensor("y", (MS, N), F32, kind="ExternalOutput")
    with tile.TileContext(nc) as tc:
        _kernel_body(tc, nc, x_in, w_in, s_in, y_out)
    nc.compile()
    return nc


def prep_inputs(x, weight, w_scale):
    """Host-side shard/layout prep. Returns in_maps for the 8 cores."""
    x = np.asarray(x)
    weight = np.asarray(weight)
    w_scale = np.asarray(w_scale, dtype=np.float32)

    # weight bytes reinterpret e4m3fn -> e4m3 exactly iff no exp-field-15 values
    wf = weight.astype(np.float32)
    assert np.abs(wf).max() <= 240.0, "weight has |v|>240; byte reinterpret invalid"
    del wf
    # wt[ch, kb, p, j] = weight[ch*CW + j, kb*B + p]
    w_prep = np.ascontiguousarray(
        weight.T.reshape(KB, B, NCH, CW).transpose(2, 0, 1, 3)
    ).view(ml_dtypes.float8_e4m3)

    # ws[p, kb, nb] = w_scale[nb, kb]
    ws_prep = np.ascontiguousarray(np.broadcast_to(w_scale.T[None], (B, KB, NB)))

    in_maps = []
    for c in range(NCORES):
        in_maps.append(
            {
                "x": np.ascontiguousarray(x[c * MS : (c + 1) * MS]),
                "wt": w_prep,
                "ws": ws_prep,
            }
        )
    return in_maps


_CACHE = {}
LAST_RESULTS = None


def kernel(x, weight, w_scale):
    global LAST_RESULTS
    if "nc" not in _CACHE:
        _CACHE["nc"] = build()
    nc = _CACHE["nc"]
    in_maps = prep_inputs(x, weight, w_scale)
    res = run_bass_kernel_spmd(
        nc,
        in_maps,
        core_ids=list(range(NCORES)),
        trace=bool(int(os.environ.get("KBQ_TRACE", "0"))),
    )
    LAST_RESULTS = res
    return np.concatenate([r["y"] for r in res.results], axis=0)


# revision 10
# speedup vs baseline: 1.6932x; 1.6932x over previous
"""Blockwise-quant linear (fp8 e4m3fn weights + per-(row,128-block) activation
quant) as a Trainium2 Bass/Tile kernel, row-parallel over 8 NeuronCores.

y[m,n] = sum_k xd[m,k] * wd[n,k], where
  xd = e4m3fn_round(x / a_s) * a_s,  a_s[m,kb] = max(amax128(x), 1e-4)/448
  wd = fp8_weight * w_scale[nb,kb]

Sharding: rows of x (M) split across cores; weight/w_scale replicated.
Each core computes y[1024, 4096] f32; host concatenates.

Device fp8 is IEEE e4m3 (max 240), reference uses e4m3fn (max 448):
 - weight bytes reinterpret exactly (values never reach exp-field-15),
 - activation quant uses half-scale: e4m3fn(v) == 2*e4m3(v/2) for |v|>2^-5.

v3 structure (single fused pipeline):
 - chunk-pair-0 GEMM is software-pipelined one m-tile behind the x-path;
   each slot's GEMM is emitted BEFORE the next x-path so the PE queue is
   [... MM(mt-1) x64, T(mt) x32 ...] and never head-of-line blocks on the
   quant chain.
 - accumulation chains are ch-outer (32 same-bank matmuls per chain) --
   bank ping-pong per matmul triggers the known PE micro-idle/HAM
   oscillation mode and measured 28% slower.
 - weight dequant runs on three engines: DVE / GpSimd tensor_tensor with
   broadcast scales, plus an ACT path using ws_all[:, kb, nb] as a
   per-partition scalar (4x [128,128] activation ops per tile).
"""

import os
from contextlib import ExitStack

import ml_dtypes
import numpy as np

import concourse.bass as bass
import concourse.mybir as mybir
import concourse.tile as tile
from concourse import bacc
from concourse.bass_utils import run_bass_kernel_spmd
from concourse.masks import make_identity

M, K, N = 8192, 4096, 4096
B = 128                 # quant block
NCORES = 8
MS = M // NCORES        # 1024 rows of x per core
KB = K // B             # 32 k-blocks
NB = N // B             # 32 n-blocks
CW = 512                # matmul moving width (1 PSUM bank of f32)
NCH = N // CW           # 8 output column chunks per core
MT = MS // B            # 8 m-tiles per core
G = 4                   # x-path column groups per m-tile
GK = KB // G            # 8 k-blocks per group

F32 = mybir.dt.float32
BF16 = mybir.dt.bfloat16
FP8 = mybir.dt.float8e4


def _kernel_body(tc, nc, x_in, w_in, s_in, y_out):
    with ExitStack() as ctx:
        consts = ctx.enter_context(tc.tile_pool(name="consts", bufs=1))
        xpool = ctx.enter_context(tc.tile_pool(name="xpool", bufs=2))
        spool = ctx.enter_context(tc.tile_pool(name="spool", bufs=2))
        xqpool = ctx.enter_context(tc.tile_pool(name="xqpool", bufs=2))
        xdpool = ctx.enter_context(tc.tile_pool(name="xdpool", bufs=2))
        xdtp = ctx.enter_context(tc.tile_pool(name="xdtp", bufs=1))
        wqpool = ctx.enter_context(tc.tile_pool(name="wqpool", bufs=12))
        wdpool = ctx.enter_context(tc.tile_pool(name="wdpool", bufs=2 * KB + 6))
        ypool = ctx.enter_context(tc.tile_pool(name="ypool", bufs=4))
        psum = ctx.enter_context(tc.tile_pool(name="psum", bufs=1, space="PSUM"))

        identity = consts.tile([B, B], BF16, name="identity")
        make_identity(nc, identity)

        # w_scale, host-expanded to [128, KB, NB] (same value on every partition)
        ws_all = consts.tile([B, KB, NB], F32, name="ws_all")
        nc.gpsimd.dma_start(ws_all[:], s_in[:])

        # resident dequantized-transposed activations: [128(k), kb, MS(m)]
        xdT = xdtp.tile([B, KB, MS], BF16, name="xdT")

        wds = {}

        def emit_w(ch, kb, eng):
            wq = wqpool.tile([B, CW], FP8, name="wq", tag="wq")
            nc.sync.dma_start(wq[:], w_in[ch, kb])
            wd = wdpool.tile([B, CW], BF16, name="wd", tag="wd")
            if eng is nc.scalar:
                # ACT path: w_scale[nb,kb] is partition-replicated in ws_all,
                # so ws_all[:, kb, nb] is a valid per-partition scalar operand
                for nb in range(CW // B):
                    i = ch * (CW // B) + nb
                    nc.scalar.mul(
                        wd[:, nb * B : (nb + 1) * B],
                        wq[:, nb * B : (nb + 1) * B],
                        ws_all[:, kb, i : i + 1],
                    )
            else:
                eng.tensor_tensor(
                    wd.rearrange("p (b j) -> p b j", j=B),
                    wq.rearrange("p (b j) -> p b j", j=B),
                    ws_all[:, kb, ch * (CW // B) : (ch + 1) * (CW // B)].broadcast_to(
                        [B, CW // B, B]
                    ),
                    op=mybir.AluOpType.mult,
                )
            wds[ch, kb] = wd

        def emit_w_pair(pair, engs, start=0, count=2 * KB):
            # ch-major to match ch-outer GEMM consumption order
            order = [(ch, kb) for ch in pair for kb in range(KB)]
            for i in range(start, min(start + count, len(order))):
                ch, kb = order[i]
                emit_w(ch, kb, engs[i % len(engs)])

        def emit_xpath(mt):
            ms = slice(mt * B, (mt + 1) * B)
            xnat = xpool.tile([B, K], BF16, name="xnat", tag="xnat")
            amax = spool.tile([B, KB], F32, name="amax", tag="amax")
            tsc = spool.tile([B, KB], F32, name="tsc", tag="tsc")
            r2 = spool.tile([B, KB], F32, name="r2", tag="r2")
            xq = xqpool.tile([B, K], FP8, name="xq", tag="xq")
            xd = xdpool.tile([B, K], BF16, name="xd", tag="xd")
            # whole-tile amax/scale ops (fewer per-op overheads on DVE)
            for g in range(G):
                nc.gpsimd.dma_start(
                    xnat[:, g * GK * B : (g + 1) * GK * B],
                    x_in[ms, g * GK * B : (g + 1) * GK * B],
                )
            nc.vector.tensor_reduce(
                amax[:], xnat.rearrange("p (b j) -> p b j", j=B),
                axis=mybir.AxisListType.X,
                op=mybir.AluOpType.max,
                apply_absolute_value=True,
            )
            # tsc = max(amax, 1e-4)/224  == 2*a_s (half-scale dequant scale)
            nc.vector.tensor_scalar(
                tsc[:], amax[:], 1e-4, 1.0 / 224.0,
                op0=mybir.AluOpType.max, op1=mybir.AluOpType.mult,
            )
            nc.vector.reciprocal(r2[:], tsc[:])
            pend = None
            for g in range(G):
                gk = slice(g * GK, (g + 1) * GK)
                gq = slice(g * GK * B, (g + 1) * GK * B)
                x3 = xnat[:, gq].rearrange("p (b j) -> p b j", j=B)
                # quantize the whole group in one op; fp8 RTNE on the store
                nc.vector.tensor_tensor(
                    xq[:, gq].rearrange("p (b j) -> p b j", j=B),
                    x3,
                    r2[:, gk].broadcast_to([B, GK, B]),
                    op=mybir.AluOpType.mult,
                )
                # dequantize on GpSimd
                nc.gpsimd.tensor_tensor(
                    xd[:, gq].rearrange("p (b j) -> p b j", j=B),
                    xq[:, gq].rearrange("p (b j) -> p b j", j=B),
                    tsc[:, gk].broadcast_to([B, GK, B]),
                    op=mybir.AluOpType.mult,
                )
                # PE-transpose GK k-blocks into one PSUM bank; wide drain copy
                # deferred one group so the engines never head-of-line block
                pst = psum.tile([B, GK * B], BF16, name="pst", tag="pst", bufs=2)
                for j in range(GK):
                    kb = g * GK + j
                    nc.tensor.transpose(
                        pst[:, j * B : (j + 1) * B],
                        xd[:, kb * B : (kb + 1) * B],
                        identity[:],
                    )
                if pend is not None:
                    nc.scalar.copy(
                        pend[1], pend[0].rearrange("p (g j) -> p g j", j=B)
                    )
                pend = (pst, xdT[:, gk, ms])
            nc.scalar.copy(pend[1], pend[0].rearrange("p (g j) -> p g j", j=B))

        def emit_gemm_ch(ch, mt):
            ms = slice(mt * B, (mt + 1) * B)
            acc = psum.tile([B, CW], F32, name="acc", tag="acc", bufs=6)
            for kb in range(KB):
                nc.tensor.matmul(
                    acc[:],
                    xdT[:, kb, ms],
                    wds[ch, kb][:],
                    start=(kb == 0),
                    stop=(kb == KB - 1),
                )
            yt = ypool.tile([B, CW], F32, name="yt", tag="yt")
            nc.scalar.copy(yt[:], acc[:])
            nc.sync.dma_start(y_out[ms, ch * CW : (ch + 1) * CW], yt[:])

        def emit_gemm(pair, mt):
            for ch in pair:
                emit_gemm_ch(ch, mt)

        # ---- phase 1: x-path software-pipelined with chunk-pair-0 GEMM.
        # Chunk 0's chains lag the x-path by 2 m-tiles, chunk 1's by 4, so
        # the pair-0 weight dequant (spread 12/slot over the first 6 slots
        # on GpSimd+ACT+DVE-spare, ch-major order) stays ahead of the
        # consumption chains. GEMM chains are emitted before xpath(mt) so
        # the PE queue never head-of-line blocks on the quant chain.
        W0 = [nc.gpsimd, nc.scalar, nc.scalar, nc.gpsimd, nc.vector]
        for mt in range(MT):
            if mt < 4:
                emit_w_pair((0, 1), W0, start=13 * mt, count=13)
            elif mt == 4:
                emit_w_pair((0, 1), W0, start=52, count=12)
            if mt >= 2:
                emit_gemm_ch(0, mt - 2)
            if mt >= 5:
                emit_gemm_ch(1, mt - 5)
            emit_xpath(mt)
        for mt in (MT - 2, MT - 1):
            emit_gemm_ch(0, mt)
        for mt in range(MT - 5, MT):
            emit_gemm_ch(1, mt)

        # ---- phase 2: remaining chunk pairs; each pair's weight dequant
        # overlaps its own GEMM (supply outruns the consumption chains) ----
        for cp in range(1, NCH // 2):
            pair = (2 * cp, 2 * cp + 1)
            emit_w_pair(pair, [nc.vector, nc.gpsimd, nc.scalar])
            for mt in range(MT):
                emit_gemm(pair, mt)


def build():
    nc = bacc.Bacc(
        "TRN2", target_bir_lowering=False, debug=False, enable_asserts=False
    )
    x_in = nc.dram_tensor("x", (MS, K), BF16, kind="ExternalInput")
    w_in = nc.dram_tensor("wt", (NCH, KB, B, CW), FP8, kind="ExternalInput")
    s_in = nc.dram_tensor("ws", (B, KB, NB), F32, kind="ExternalInput")
    y_out = nc.dram_tensor("y", (MS, N), F32, kind="ExternalOutput")
    with tile.TileContext(nc) as tc:
        _kernel_body(tc, nc, x_in, w_in, s_in, y_out)
    nc.compile()
    return nc


def prep_inputs(x, weight, w_scale):
    """Host-side shard/layout prep. Returns in_maps for the 8 cores."""
    x = np.asarray(x)
    weight = np.asarray(weight)
    w_scale = np.asarray(w_scale, dtype=np.float32)

    # weight bytes reinterpret e4m3fn -> e4m3 exactly iff no exp-field-15 values
    wf = weight.astype(np.float32)
    assert np.abs(wf).max() <= 240.0, "weight has |v|>240; byte reinterpret invalid"
    del wf
    # wt[ch, kb, p, j] = weight[ch*CW + j, kb*B + p]
    w_prep = np.ascontiguousarray(
        weight.T.reshape(KB, B, NCH, CW).transpose(2, 0, 1, 3)
    ).view(ml_dtypes.float8_e4m3)

    # ws[p, kb, nb] = w_scale[nb, kb]
    ws_prep = np.ascontiguousarray(np.broadcast_to(w_scale.T[None], (B, KB, NB)))

    in_maps = []
    for c in range(NCORES):
        in_maps.append(
            {
                "x": np.ascontiguousarray(x[c * MS : (c + 1) * MS]),
                "wt": w_prep,
                "ws": ws_prep,
            }
        )
    return in_maps


_CACHE = {}
LAST_RESULTS = None


def kernel(x, weight, w_scale):
    global LAST_RESULTS
    if "nc" not in _CACHE:
        _CACHE["nc"] = build()
    nc = _CACHE["nc"]
    in_maps = prep_inputs(x, weight, w_scale)
    res = run_bass_kernel_spmd(
        nc,
        in_maps,
        core_ids=list(range(NCORES)),
        trace=bool(int(os.environ.get("KBQ_TRACE", "0"))),
    )
    LAST_RESULTS = res
    return np.concatenate([r["y"] for r in res.results], axis=0)


# revision 16
# speedup vs baseline: 1.7441x; 1.0301x over previous
"""Blockwise-quant linear (fp8 e4m3fn weights + per-(row,128-block) activation
quant) as a Trainium2 Bass/Tile kernel, row-parallel over 8 NeuronCores.

y[m,n] = sum_k xd[m,k] * wd[n,k], where
  xd = e4m3fn_round(x / a_s) * a_s,  a_s[m,kb] = max(amax128(x), 1e-4)/448
  wd = fp8_weight * w_scale[nb,kb]

Sharding: rows of x (M) split across cores; weight/w_scale replicated.
Each core computes y[1024, 4096] f32; host concatenates.

Device fp8 is IEEE e4m3 (max 240), reference uses e4m3fn (max 448):
 - weight bytes reinterpret exactly (values never reach exp-field-15),
 - activation quant uses half-scale: e4m3fn(v) == 2*e4m3(v/2) for |v|>2^-5.

v3 structure (single fused pipeline):
 - chunk-pair-0 GEMM is software-pipelined one m-tile behind the x-path;
   each slot's GEMM is emitted BEFORE the next x-path so the PE queue is
   [... MM(mt-1) x64, T(mt) x32 ...] and never head-of-line blocks on the
   quant chain.
 - accumulation chains are ch-outer (32 same-bank matmuls per chain) --
   bank ping-pong per matmul triggers the known PE micro-idle/HAM
   oscillation mode and measured 28% slower.
 - weight dequant runs on three engines: DVE / GpSimd tensor_tensor with
   broadcast scales, plus an ACT path using ws_all[:, kb, nb] as a
   per-partition scalar (4x [128,128] activation ops per tile).
"""

import os
from contextlib import ExitStack

import ml_dtypes
import numpy as np

import concourse.bass as bass
import concourse.mybir as mybir
import concourse.tile as tile
from concourse import bacc
from concourse.bass_utils import run_bass_kernel_spmd
from concourse.masks import make_identity

M, K, N = 8192, 4096, 4096
B = 128                 # quant block
NCORES = 8
MS = M // NCORES        # 1024 rows of x per core
KB = K // B             # 32 k-blocks
NB = N // B             # 32 n-blocks
CW = 512                # matmul moving width (1 PSUM bank of f32)
NCH = N // CW           # 8 output column chunks per core
MT = MS // B            # 8 m-tiles per core
G = 4                   # x-path column groups per m-tile
GK = KB // G            # 8 k-blocks per group

F32 = mybir.dt.float32
BF16 = mybir.dt.bfloat16
FP8 = mybir.dt.float8e4


def _drain(nc, pend):
    # alternate xdT drains between DVE and ACT to balance phase-1 load
    pst, dst, g = pend
    src = pst.rearrange("p (g j) -> p g j", j=B)
    if g % 2:
        nc.vector.tensor_copy(dst, src)
    else:
        nc.scalar.copy(dst, src)


def _kernel_body(tc, nc, x_in, w_in, s_in, y_out):
    with ExitStack() as ctx:
        consts = ctx.enter_context(tc.tile_pool(name="consts", bufs=1))
        xpool = ctx.enter_context(tc.tile_pool(name="xpool", bufs=3))
        spool = ctx.enter_context(tc.tile_pool(name="spool", bufs=2))
        xqpool = ctx.enter_context(tc.tile_pool(name="xqpool", bufs=2))
        xdpool = ctx.enter_context(tc.tile_pool(name="xdpool", bufs=2))
        xdtp = ctx.enter_context(tc.tile_pool(name="xdtp", bufs=1))
        wqpool = ctx.enter_context(tc.tile_pool(name="wqpool", bufs=12))
        wdpool = ctx.enter_context(tc.tile_pool(name="wdpool", bufs=2 * KB + 6))
        ypool = ctx.enter_context(tc.tile_pool(name="ypool", bufs=4))
        psum = ctx.enter_context(tc.tile_pool(name="psum", bufs=1, space="PSUM"))

        identity = consts.tile([B, B], BF16, name="identity")
        make_identity(nc, identity)

        # w_scale, host-expanded to [128, KB, NB] (same value on every partition)
        ws_all = consts.tile([B, KB, NB], F32, name="ws_all")
        nc.gpsimd.dma_start(ws_all[:], s_in[:])

        # resident dequantized-transposed activations: [128(k), kb, MS(m)]
        xdT = xdtp.tile([B, KB, MS], BF16, name="xdT")

        wds = {}

        def emit_w(ch, kb, eng):
            wq = wqpool.tile([B, CW], FP8, name="wq", tag="wq")
            nc.sync.dma_start(wq[:], w_in[ch, kb])
            wd = wdpool.tile([B, CW], BF16, name="wd", tag="wd")
            if eng is nc.scalar:
                # ACT path: w_scale[nb,kb] is partition-replicated in ws_all,
                # so ws_all[:, kb, nb] is a valid per-partition scalar operand
                for nb in range(CW // B):
                    i = ch * (CW // B) + nb
                    nc.scalar.mul(
                        wd[:, nb * B : (nb + 1) * B],
                        wq[:, nb * B : (nb + 1) * B],
                        ws_all[:, kb, i : i + 1],
                    )
            else:
                eng.tensor_tensor(
                    wd.rearrange("p (b j) -> p b j", j=B),
                    wq.rearrange("p (b j) -> p b j", j=B),
                    ws_all[:, kb, ch * (CW // B) : (ch + 1) * (CW // B)].broadcast_to(
                        [B, CW // B, B]
                    ),
                    op=mybir.AluOpType.mult,
                )
            wds[ch, kb] = wd

        def emit_w_pair(pair, engs, start=0, count=2 * KB):
            # ch-major to match ch-outer GEMM consumption order
            order = [(ch, kb) for ch in pair for kb in range(KB)]
            for i in range(start, min(start + count, len(order))):
                ch, kb = order[i]
                emit_w(ch, kb, engs[i % len(engs)])

        xnats = {}

        def emit_xdma(mt):
            ms = slice(mt * B, (mt + 1) * B)
            xnat = xpool.tile([B, K], BF16, name="xnat", tag="xnat")
            for g in range(G):
                nc.gpsimd.dma_start(
                    xnat[:, g * GK * B : (g + 1) * GK * B],
                    x_in[ms, g * GK * B : (g + 1) * GK * B],
                )
            xnats[mt] = xnat

        def emit_xpath(mt):
            ms = slice(mt * B, (mt + 1) * B)
            xnat = xnats.pop(mt)
            amax = spool.tile([B, KB], F32, name="amax", tag="amax")
            tsc = spool.tile([B, KB], F32, name="tsc", tag="tsc")
            r2 = spool.tile([B, KB], F32, name="r2", tag="r2")
            xq = xqpool.tile([B, K], FP8, name="xq", tag="xq")
            xd = xdpool.tile([B, K], BF16, name="xd", tag="xd")
            # whole-tile amax/scale ops (fewer per-op overheads on DVE)
            nc.vector.tensor_reduce(
                amax[:], xnat.rearrange("p (b j) -> p b j", j=B),
                axis=mybir.AxisListType.X,
                op=mybir.AluOpType.max,
                apply_absolute_value=True,
            )
            # tsc = max(amax, 1e-4)/224  == 2*a_s (half-scale dequant scale)
            nc.vector.tensor_scalar(
                tsc[:], amax[:], 1e-4, 1.0 / 224.0,
                op0=mybir.AluOpType.max, op1=mybir.AluOpType.mult,
            )
            nc.vector.reciprocal(r2[:], tsc[:])
            pend = None
            for g in range(G):
                gk = slice(g * GK, (g + 1) * GK)
                gq = slice(g * GK * B, (g + 1) * GK * B)
                x3 = xnat[:, gq].rearrange("p (b j) -> p b j", j=B)
                # quantize the whole group in one op on GpSimd; fp8 RTNE on
                # the store (DVE is the phase-1 critical engine: it keeps
                # amax + most of the pair-0 weight dequant)
                nc.gpsimd.tensor_tensor(
                    xq[:, gq].rearrange("p (b j) -> p b j", j=B),
                    x3,
                    r2[:, gk].broadcast_to([B, GK, B]),
                    op=mybir.AluOpType.mult,
                )
                # dequantize per k-block on ACT (tsc is per-partition there)
                for j in range(GK):
                    kb = g * GK + j
                    kc = slice(kb * B, (kb + 1) * B)
                    nc.scalar.mul(xd[:, kc], xq[:, kc], tsc[:, kb : kb + 1])
                # PE-transpose GK k-blocks into one PSUM bank; wide drain copy
                # deferred one group so the engines never head-of-line block
                pst = psum.tile([B, GK * B], BF16, name="pst", tag="pst", bufs=2)
                for j in range(GK):
                    kb = g * GK + j
                    nc.tensor.transpose(
                        pst[:, j * B : (j + 1) * B],
                        xd[:, kb * B : (kb + 1) * B],
                        identity[:],
                    )
                if pend is not None:
                    _drain(nc, pend)
                pend = (pst, xdT[:, gk, ms], g)
            _drain(nc, pend)

        def emit_gemm_ch(ch, mt):
            ms = slice(mt * B, (mt + 1) * B)
            acc = psum.tile([B, CW], F32, name="acc", tag="acc", bufs=6)
            for kb in range(KB):
                nc.tensor.matmul(
                    acc[:],
                    xdT[:, kb, ms],
                    wds[ch, kb][:],
                    start=(kb == 0),
                    stop=(kb == KB - 1),
                )
            yt = ypool.tile([B, CW], F32, name="yt", tag="yt")
            nc.scalar.copy(yt[:], acc[:])
            nc.sync.dma_start(y_out[ms, ch * CW : (ch + 1) * CW], yt[:])

        def emit_gemm(pair, mt):
            for ch in pair:
                emit_gemm_ch(ch, mt)

        # ---- phase 1: x-path software-pipelined with chunk-pair-0 GEMM.
        # Chunk 0's chains lag the x-path by 2 m-tiles, chunk 1's by 4, so
        # the pair-0 weight dequant (mostly on DVE; front-loaded 6 tiles in
        # the dead time before x lands, then 14/slot, ch-major) stays ahead
        # of the consumption chains. GEMM chains are emitted before
        # xpath(mt) so the PE queue never head-of-line blocks.
        W0 = [nc.vector, nc.vector, nc.vector, nc.gpsimd, nc.vector,
              nc.vector, nc.scalar]
        emit_xdma(0)
        emit_xdma(1)
        emit_w_pair((0, 1), W0, start=0, count=6)
        for mt in range(MT):
            if mt < 4:
                emit_w_pair((0, 1), W0, start=6 + 15 * mt, count=15)
            if mt >= 2:
                emit_gemm_ch(0, mt - 2)
            if mt >= 4:
                emit_gemm_ch(1, mt - 4)
            emit_xpath(mt)
            if mt + 2 < MT:
                emit_xdma(mt + 2)
        for mt in (MT - 2, MT - 1):
            emit_gemm_ch(0, mt)
        for mt in range(MT - 4, MT):
            emit_gemm_ch(1, mt)

        # ---- phase 2: remaining chunk pairs; each pair's weight dequant
        # overlaps its own GEMM (supply outruns the consumption chains) ----
        for cp in range(1, NCH // 2):
            pair = (2 * cp, 2 * cp + 1)
            emit_w_pair(pair, [nc.vector, nc.gpsimd, nc.scalar])
            for mt in range(MT):
                emit_gemm(pair, mt)


def build():
    nc = bacc.Bacc(
        "TRN2", target_bir_lowering=False, debug=False, enable_asserts=False
    )
    x_in = nc.dram_tensor("x", (MS, K), BF16, kind="ExternalInput")
    w_in = nc.dram_tensor("wt", (NCH, KB, B, CW), FP8, kind="ExternalInput")
    s_in = nc.dram_tensor("ws", (B, KB, NB), F32, kind="ExternalInput")
    y_out = nc.dram_tensor("y", (MS, N), F32, kind="ExternalOutput")
    with tile.TileContext(nc) as tc:
        _kernel_body(tc, nc, x_in, w_in, s_in, y_out)
    nc.compile()
    return nc


def prep_inputs(x, weight, w_scale):
    """Host-side shard/layout prep. Returns in_maps for the 8 cores."""
    x = np.asarray(x)
    weight = np.asarray(weight)
    w_scale = np.asarray(w_scale, dtype=np.float32)

    # weight bytes reinterpret e4m3fn -> e4m3 exactly iff no exp-field-15 values
    wf = weight.astype(np.float32)
    assert np.abs(wf).max() <= 240.0, "weight has |v|>240; byte reinterpret invalid"
    del wf
    # wt[ch, kb, p, j] = weight[ch*CW + j, kb*B + p]
    w_prep = np.ascontiguousarray(
        weight.T.reshape(KB, B, NCH, CW).transpose(2, 0, 1, 3)
    ).view(ml_dtypes.float8_e4m3)

    # ws[p, kb, nb] = w_scale[nb, kb]
    ws_prep = np.ascontiguousarray(np.broadcast_to(w_scale.T[None], (B, KB, NB)))

    in_maps = []
    for c in range(NCORES):
        in_maps.append(
            {
                "x": np.ascontiguousarray(x[c * MS : (c + 1) * MS]),
                "wt": w_prep,
                "ws": ws_prep,
            }
        )
    return in_maps


_CACHE = {}
LAST_RESULTS = None


def kernel(x, weight, w_scale):
    global LAST_RESULTS
    if "nc" not in _CACHE:
        _CACHE["nc"] = build()
    nc = _CACHE["nc"]
    in_maps = prep_inputs(x, weight, w_scale)
    res = run_bass_kernel_spmd(
        nc,
        in_maps,
        core_ids=list(range(NCORES)),
        trace=bool(int(os.environ.get("KBQ_TRACE", "0"))),
    )
    LAST_RESULTS = res
    return np.concatenate([r["y"] for r in res.results], axis=0)
